# revision 3
# baseline (speedup 1.0000x reference)
"""Trainium2 Bass kernel for nn_BroadcastHub (MoE routing hub).

Strategy (8 NeuronCores, data-parallel over batch):
  - Each core owns B/8 = 512 rows of input_data (shipped transposed, bf16).
  - Gate logits are computed FIRST via the algebraic identity
        spec[b,e,:] @ Wg[e] + bg[e] = x[b] @ (Ws[e].T @ Wg[e]) + (bs[e]@Wg[e] + bg[e])
    with V = Ws.T @ Wg precomputed on the host from weights only.  This gives
    the softmax gate weights gw[b,e] before any expert GEMM runs, so
    combined = sum_e gw[b,e] * (x @ Ws[e].T + bs[e]) accumulates into an SBUF
    fp32 buffer expert-by-expert, never materializing spec[B,E,d].
  - Expert weights stream through SBUF once per core (bf16, 268 MB).
  - The level-net (combined -> relu -> logits) runs incrementally per
    256-column chunk of combined, via PE transpose.
  - err/probs statistics are summed per-core and AllReduced across the 8
    cores (tiny [1,8] collective), so level semantics match the reference
    globally; the a*combined + b*hub mix happens on device.
"""

import sys

for _p in ("/opt/trn_rl_repo",):
    if _p not in sys.path:
        sys.path.insert(0, _p)

import numpy as np
import ml_dtypes

import concourse.bass as bass
import concourse.tile as tile
from concourse import bacc, mybir
from concourse.bass_utils import run_bass_kernel_spmd
from concourse.masks import make_identity

BF16 = mybir.dt.bfloat16
F32 = mybir.dt.float32
I32 = mybir.dt.int32
AX = mybir.AxisListType
ALU = mybir.AluOpType
ACTF = mybir.ActivationFunctionType
ts = bass.ts

NCORES = 8
P = 128

CUTOFF = 1.5
FADE = 0.9


def build_program(B=4096, D=4096, E=8, DL1=2048, DE1=1024, NL=4, OC=256):
    SH = B // NCORES          # rows per core
    NBT = SH // P             # b-tiles per core
    KO = D // P               # contraction subtiles
    NOC = D // OC             # output-column chunks
    OSUB = OC // P            # 128-subtiles per chunk
    NJT = DL1 // P            # level-net hidden tiles
    CH = 512                  # generic free-dim chunk
    NCH = D // CH

    nc = bacc.Bacc("TRN2", target_bir_lowering=False, debug=False,
                   num_devices=NCORES)

    # ---- dram parameters (per core) ----
    xT_d = nc.declare_dram_parameter("xT", [D, SH], BF16, isOutput=False)
    wsT_d = nc.declare_dram_parameter("wsT", [E, D, D], BF16, isOutput=False)
    v_d = nc.declare_dram_parameter("v", [D, E], BF16, isOutput=False)
    ce_d = nc.declare_dram_parameter("ce", [1, E], BF16, isOutput=False)
    bs_d = nc.declare_dram_parameter("bs", [E, D], BF16, isOutput=False)
    wl1T_d = nc.declare_dram_parameter("wl1T", [D, DL1], BF16, isOutput=False)
    wl2T_d = nc.declare_dram_parameter("wl2T", [DL1, NL], BF16, isOutput=False)
    bl1_d = nc.declare_dram_parameter("bl1r", [P, NJT], F32, isOutput=False)
    bl2_d = nc.declare_dram_parameter("bl2r", [1, NL], BF16, isOutput=False)
    we1T_d = nc.declare_dram_parameter("we1T", [D, DE1], BF16, isOutput=False)
    we2_d = nc.declare_dram_parameter("we2r", [1, DE1], BF16, isOutput=False)
    be1_d = nc.declare_dram_parameter("be1r", [1, DE1], F32, isOutput=False)
    be2_d = nc.declare_dram_parameter("be2s", [1, 1], F32, isOutput=False)
    hubr_d = nc.declare_dram_parameter("hubr", [1, D], BF16, isOutput=False)
    hubt_d = nc.declare_dram_parameter("hubT", [D, 1], BF16, isOutput=False)

    out_d = nc.declare_dram_parameter("out", [SH, D], F32, isOutput=True)
    lvl_d = nc.declare_dram_parameter("lvl", [1, 1], I32, isOutput=True)
    st_d = nc.declare_dram_parameter("stats", [1, 16], F32, isOutput=True)

    cc_in_t = nc.dram_tensor("cc_in", [1, 8], F32)
    cc_out_t = nc.dram_tensor("cc_out", [1, 8], F32)

    with tile.TileContext(nc) as tc:
        with (
            tc.tile_pool(name="consts", bufs=1) as consts,
            tc.tile_pool(name="big", bufs=1) as big,
            tc.tile_pool(name="accp", bufs=NBT) as accp,
            tc.tile_pool(name="gwp", bufs=NBT) as gwp,
            tc.tile_pool(name="wpool", bufs=2) as wpool,
            tc.tile_pool(name="wl1p", bufs=2) as wl1p,
            tc.tile_pool(name="we1p", bufs=2) as we1p,
            tc.tile_pool(name="tp", bufs=2) as tp,
            tc.tile_pool(name="ctp", bufs=2 * OSUB) as ctp,
            tc.tile_pool(name="sp", bufs=1) as sp,
            tc.tile_pool(name="lp", bufs=3) as lp,
            tc.tile_pool(name="ps_mm", bufs=4, space="PSUM") as ps_mm,
            tc.tile_pool(name="ps_h1", bufs=2, space="PSUM") as ps_h1,
            tc.tile_pool(name="ps_ms", bufs=2, space="PSUM") as ps_ms,
            tc.tile_pool(name="dram", bufs=1, space="DRAM") as _dram,
        ):
            # ---------------- constants ----------------
            ident = consts.tile([P, P], BF16)
            make_identity(nc, ident)
            ones_row = consts.tile([1, P], BF16)   # lhsT for partition-bcast
            nc.vector.memset(ones_row, 1.0)
            ones_col = consts.tile([P, 1], BF16)   # lhsT for partition-sum
            nc.vector.memset(ones_col, 1.0)
            iota_i = consts.tile([1, NL], I32)
            nc.gpsimd.iota(iota_i, pattern=[[1, NL]], base=0, channel_multiplier=0)
            iota_f = consts.tile([1, NL], F32)
            nc.vector.tensor_copy(out=iota_f, in_=iota_i)

            xt = big.tile([P, KO, SH], BF16, tag="xt")
            nc.sync.dma_start(out=xt, in_=xT_d.rearrange("(ko ki) b -> ki ko b", ki=P))
            vt = consts.tile([P, KO, E], BF16)
            nc.sync.dma_start(out=vt, in_=v_d.rearrange("(ko ki) e -> ki ko e", ki=P))
            ce_sb = consts.tile([1, E], BF16)
            nc.sync.dma_start(out=ce_sb, in_=ce_d[:, :])
            wl2t = consts.tile([P, NJT, NL], BF16)
            nc.sync.dma_start(out=wl2t, in_=wl2T_d.rearrange("(ko ki) l -> ki ko l", ki=P))
            bl1_sb = consts.tile([P, NJT], F32)
            nc.sync.dma_start(out=bl1_sb, in_=bl1_d[:, :])
            bl2_sb = consts.tile([1, NL], BF16)
            nc.sync.dma_start(out=bl2_sb, in_=bl2_d[:, :])
            we2r = consts.tile([1, DE1], BF16)
            nc.sync.dma_start(out=we2r, in_=we2_d[:, :])
            be1r = consts.tile([1, DE1], F32)
            nc.sync.dma_start(out=be1r, in_=be1_d[:, :])
            be2s = consts.tile([1, 1], F32)
            nc.sync.dma_start(out=be2s, in_=be2_d[:, :])
            hubr = consts.tile([1, D], BF16)
            nc.sync.dma_start(out=hubr, in_=hubr_d[:, :])
            hubt = consts.tile([P, KO, 1], BF16)
            nc.sync.dma_start(out=hubt, in_=hubt_d.rearrange("(ko ki) o -> ki ko o", ki=P))

            # persistent accumulators / gates
            acc = [accp.tile([P, D], F32, tag="acc", name=f"acc{_b}") for _b in range(NBT)]
            gw = [gwp.tile([P, E], F32, tag="gw", name=f"gw{_b}") for _b in range(NBT)]
            gwT = big.tile([E, SH], BF16, tag="gwT")
            h1sb = big.tile([P, NJT, SH], BF16, tag="h1sb")

            # ---------------- phase: error-net scalar c ----------------
            with nc.named_scope("prep_c"):
                NEC = (DE1 + CH - 1) // CH
                he_row = sp.tile([1, DE1], F32, tag="he_row")
                for c in range(NEC):
                    cw = min(CH, DE1 - c * CH)
                    ps_he = ps_ms.tile([1, CH], F32, tag="ps")
                    for k in range(KO):
                        wek = we1p.tile([P, CH], BF16, tag="we1k")
                        nc.sync.dma_start(
                            out=wek[:, :cw],
                            in_=we1T_d.rearrange("(ko ki) j -> ki ko j", ki=P)[
                                :, k, c * CH : c * CH + cw],
                        )
                        nc.tensor.matmul(ps_he[:, :cw], hubt[:, k, :], wek[:, :cw],
                                         start=(k == 0), stop=(k == KO - 1))
                    nc.vector.tensor_tensor(
                        out=he_row[:, c * CH : c * CH + cw], in0=ps_he[:, :cw],
                        in1=be1r[:, c * CH : c * CH + cw], op=ALU.add)
                he_bf = sp.tile([1, DE1], BF16, tag="he_bf")
                nc.scalar.activation(out=he_bf, in_=he_row, func=ACTF.Tanh)
                hw_prod = sp.tile([1, DE1], F32, tag="he_row", name="hw_prod")
                nc.vector.tensor_mul(hw_prod, he_bf, we2r)
                c_t = sp.tile([1, 1], F32, tag="c_t")
                nc.vector.reduce_sum(out=c_t, in_=hw_prod, axis=AX.X)
                nc.vector.tensor_tensor(out=c_t, in0=c_t, in1=be2s, op=ALU.add)

            # ---------------- phase: per-row err stats ----------------
            with nc.named_scope("err_stats"):
                ps_s1 = ps_ms.tile([1, SH], F32, tag="ps")
                ps_s2 = ps_h1.tile([1, SH], F32, tag="h1")
                for k in range(KO):
                    sq = tp.tile([P, SH], BF16, tag="sq")
                    nc.vector.tensor_mul(sq, xt[:, k, :], xt[:, k, :])
                    nc.tensor.matmul(ps_s1, ones_col, xt[:, k, :],
                                     start=(k == 0), stop=(k == KO - 1))
                    nc.tensor.matmul(ps_s2, ones_col, sq,
                                     start=(k == 0), stop=(k == KO - 1))
                s1r = sp.tile([1, SH], F32, tag="s1r")
                nc.vector.tensor_copy(out=s1r, in_=ps_s1)
                s2r = sp.tile([1, SH], F32, tag="s2r")
                nc.vector.tensor_copy(out=s2r, in_=ps_s2)
                # recon = S2 - 2c*S1 + D*c^2 ; ratio = 0.5*sqrt(S2)/(sqrt(D)*|c|+eps)
                m2c = sp.tile([1, 1], F32, tag="m2c")
                nc.vector.tensor_scalar(out=m2c, in0=c_t, scalar1=-2.0, scalar2=None,
                                        op0=ALU.mult)
                ccd = sp.tile([1, 1], F32, tag="ccd")
                nc.vector.tensor_mul(ccd, c_t, c_t)
                nc.vector.tensor_scalar(out=ccd, in0=ccd, scalar1=float(D),
                                        scalar2=None, op0=ALU.mult)
                emag = sp.tile([1, 1], F32, tag="emag")
                nc.scalar.activation(out=emag, in_=c_t, func=ACTF.Abs)
                nc.vector.tensor_scalar(out=emag, in0=emag,
                                        scalar1=float(np.sqrt(D)), scalar2=1e-7,
                                        op0=ALU.mult, op1=ALU.add)
                remag = sp.tile([1, 1], F32, tag="remag")
                nc.vector.reciprocal(out=remag, in_=emag)
                # erow (in s1r): S1*m2c + S2 + D*c^2 ; mag (in s2r): 0.5*sqrt(S2)/emag
                nc.vector.scalar_tensor_tensor(out=s1r, in0=s1r, scalar=m2c[:, 0:1],
                                               in1=s2r, op0=ALU.mult, op1=ALU.add)
                nc.vector.tensor_scalar(out=s1r, in0=s1r, scalar1=ccd[:, 0:1],
                                        scalar2=None, op0=ALU.add)
                nc.scalar.activation(out=s2r, in_=s2r, func=ACTF.Sqrt)
                nc.vector.tensor_scalar(out=s2r, in0=s2r, scalar1=remag[:, 0:1],
                                        scalar2=0.5, op0=ALU.mult, op1=ALU.mult)
                nc.vector.tensor_tensor(out=s1r, in0=s1r, in1=s2r, op=ALU.add)
                err_p = sp.tile([1, 1], F32, tag="err_p")
                nc.vector.reduce_sum(out=err_p, in_=s1r, axis=AX.X)

            # ---------------- phase: gates ----------------
            with nc.named_scope("gates"):
                ps_ceb = ps_ms.tile([P, E], F32, tag="ps")
                nc.tensor.matmul(ps_ceb, ones_row, ce_sb, start=True, stop=True)
                ce_b = sp.tile([P, E], F32, tag="ce_b")
                nc.vector.tensor_copy(out=ce_b, in_=ps_ceb)
                for bt in range(NBT):
                    ps_g = ps_ms.tile([P, E], F32, tag="ps")
                    for k in range(KO):
                        nc.tensor.matmul(ps_g, xt[:, k, ts(bt, P)], vt[:, k, :],
                                         start=(k == 0), stop=(k == KO - 1))
                    nc.vector.tensor_tensor(out=ps_g, in0=ps_g, in1=ce_b, op=ALU.add)
                    gate = lp.tile([P, E], F32, tag="gate")
                    nc.scalar.activation(out=gate, in_=ps_g, func=ACTF.Sigmoid)
                    mx = lp.tile([P, 1], F32, tag="gmx")
                    nc.vector.reduce_max(out=mx, in_=gate, axis=AX.X)
                    nmx = lp.tile([P, 1], F32, tag="gnmx")
                    nc.vector.tensor_scalar(out=nmx, in0=mx, scalar1=-1.0,
                                            scalar2=None, op0=ALU.mult)
                    ex = lp.tile([P, E], F32, tag="gex")
                    nc.scalar.activation(out=ex, in_=gate, func=ACTF.Exp,
                                         bias=nmx[:, 0:1], scale=1.0)
                    sm = lp.tile([P, 1], F32, tag="gsm")
                    nc.vector.reduce_sum(out=sm, in_=ex, axis=AX.X)
                    rsm = lp.tile([P, 1], F32, tag="grsm")
                    nc.vector.reciprocal(out=rsm, in_=sm)
                    nc.vector.tensor_scalar(out=gw[bt], in0=ex, scalar1=rsm[:, 0:1],
                                            scalar2=None, op0=ALU.mult)
                    gwbf = lp.tile([P, E], BF16, tag="gwbf")
                    nc.vector.tensor_copy(out=gwbf, in_=gw[bt])
                    ps_t = ps_ms.tile([E, P], BF16, tag="ps")
                    nc.tensor.transpose(ps_t, gwbf, ident)
                    nc.vector.tensor_copy(out=gwT[:, ts(bt, P)], in_=ps_t)

            # ---------------- phase: acc init with gw @ bs ----------------
            with nc.named_scope("acc_init"):
                for ch in range(NCH):
                    bs_ch = tp.tile([E, CH], BF16, tag="sq", name=f"bs_ch{ch}")
                    nc.sync.dma_start(out=bs_ch, in_=bs_d[:, ts(ch, CH)])
                    for bt in range(NBT):
                        ps_b = ps_h1.tile([P, CH], F32, tag="h1")
                        nc.tensor.matmul(ps_b, gwT[:, ts(bt, P)],
                                         bs_ch, start=True, stop=True)
                        nc.vector.tensor_copy(out=acc[bt][:, ts(ch, CH)], in_=ps_b)

            # ---------------- phase: main expert GEMMs + level-net h1 ------
            nc.vector.memset(h1sb, 0.0)
            for oc in range(NOC):
                with nc.named_scope("moe"):
                    for e in range(E):
                        wt = wpool.tile([P, KO, OC], BF16, tag="wt")
                        nc.sync.dma_start(
                            out=wt,
                            in_=wsT_d[e].rearrange("(ko ki) o -> ki ko o", ki=P)[
                                :, :, ts(oc, OC)],
                        )
                        for bt in range(NBT):
                            ps = ps_mm.tile([P, OC], F32, tag="mm")
                            for k in range(KO):
                                nc.tensor.matmul(ps, xt[:, k, ts(bt, P)], wt[:, k, :],
                                                 start=(k == 0), stop=(k == KO - 1))
                            nc.vector.scalar_tensor_tensor(
                                out=acc[bt][:, ts(oc, OC)], in0=ps,
                                scalar=gw[bt][:, e:e + 1],
                                in1=acc[bt][:, ts(oc, OC)],
                                op0=ALU.mult, op1=ALU.add)
                with nc.named_scope("lvl_h1"):
                    # transpose this finished chunk of combined -> cT (bf16)
                    ct = [ctp.tile([P, SH], BF16, tag="ct", name=f"ct{_o}") for _o in range(OSUB)]
                    for bt in range(NBT):
                        cbf = tp.tile([P, OC], BF16, tag="sq")
                        nc.vector.tensor_copy(out=cbf, in_=acc[bt][:, ts(oc, OC)])
                        for os_ in range(OSUB):
                            ps_t = ps_ms.tile([P, P], BF16, tag="ps")
                            nc.tensor.transpose(ps_t, cbf[:, ts(os_, P)], ident)
                            nc.vector.tensor_copy(out=ct[os_][:, ts(bt, P)], in_=ps_t)
                    wl1t = wl1p.tile([P, OSUB, DL1], BF16, tag="wl1t")
                    nc.sync.dma_start(
                        out=wl1t,
                        in_=wl1T_d[ts(oc, OC), :].rearrange(
                            "(os ki) j -> ki os j", ki=P),
                    )
                    for jt in range(NJT):
                        ps_h = ps_h1.tile([P, SH], F32, tag="h1")
                        for os_ in range(OSUB):
                            nc.tensor.matmul(ps_h, wl1t[:, os_, ts(jt, P)], ct[os_],
                                             start=(os_ == 0), stop=(os_ == OSUB - 1))
                        nc.vector.tensor_tensor(out=h1sb[:, jt, :], in0=ps_h,
                                                in1=h1sb[:, jt, :], op=ALU.add)

            # ---------------- phase: level-net logits + probs ----------------
            with nc.named_scope("lvl_logits"):
                ps_bl2 = ps_ms.tile([P, NL], F32, tag="ps")
                nc.tensor.matmul(ps_bl2, ones_row, bl2_sb, start=True, stop=True)
                bl2_b = sp.tile([P, NL], F32, tag="bl2_b")
                nc.vector.tensor_copy(out=bl2_b, in_=ps_bl2)
                lg = [ps_mm.tile([P, NL], F32, tag="mm", name=f"lg{_b}") for _b in range(NBT)]
                for jt in range(NJT):
                    h1bf = tp.tile([P, SH], BF16, tag="h1bf")
                    nc.scalar.activation(out=h1bf, in_=h1sb[:, jt, :], func=ACTF.Relu,
                                         bias=bl1_sb[:, jt:jt + 1], scale=1.0)
                    for bt in range(NBT):
                        nc.tensor.matmul(lg[bt], h1bf[:, ts(bt, P)], wl2t[:, jt, :],
                                         start=(jt == 0), stop=(jt == NJT - 1))
                ps_pp = ps_ms.tile([1, NL], F32, tag="ps")
                for bt in range(NBT):
                    lgs = lp.tile([P, NL], F32, tag="lgs")
                    nc.vector.tensor_tensor(out=lgs, in0=lg[bt], in1=bl2_b, op=ALU.add)
                    lmx = lp.tile([P, 1], F32, tag="lmx")
                    nc.vector.reduce_max(out=lmx, in_=lgs, axis=AX.X)
                    nlmx = lp.tile([P, 1], F32, tag="nlmx")
                    nc.vector.tensor_scalar(out=nlmx, in0=lmx, scalar1=-1.0,
                                            scalar2=None, op0=ALU.mult)
                    lex = lp.tile([P, NL], F32, tag="lex")
                    nc.scalar.activation(out=lex, in_=lgs, func=ACTF.Exp,
                                         bias=nlmx[:, 0:1], scale=1.0)
                    lsm = lp.tile([P, 1], F32, tag="lsm")
                    nc.vector.reduce_sum(out=lsm, in_=lex, axis=AX.X)
                    lrs = lp.tile([P, 1], F32, tag="lrs")
                    nc.vector.reciprocal(out=lrs, in_=lsm)
                    prb = lp.tile([P, NL], BF16, tag="prb")
                    nc.vector.tensor_scalar(out=prb, in0=lex, scalar1=lrs[:, 0:1],
                                            scalar2=None, op0=ALU.mult)
                    nc.tensor.matmul(ps_pp, ones_col, prb,
                                     start=(bt == 0), stop=(bt == NBT - 1))

            # ---------------- phase: AllReduce stats ----------------
            with nc.named_scope("cc"):
                cc_sb = sp.tile([1, 8], F32, tag="cc_sb")
                nc.vector.memset(cc_sb, 0.0)
                nc.vector.tensor_copy(out=cc_sb[:, 0:1], in_=err_p)
                nc.vector.tensor_copy(out=cc_sb[:, 1:1 + NL], in_=ps_pp)
                nc.sync.dma_start(out=cc_in_t[:, :], in_=cc_sb)
                nc.gpsimd.collective_compute(
                    "AllReduce", ALU.add,
                    replica_groups=[list(range(NCORES))],
                    ins=[cc_in_t[:, :].opt()],
                    outs=[cc_out_t[:, :].opt()],
                )
                cc_res = sp.tile([1, 8], F32, tag="cc_res")
                nc.sync.dma_start(out=cc_res, in_=cc_out_t[:, :])

            # ---------------- phase: level decision ----------------
            with nc.named_scope("level"):
                inv_b = 1.0 / float(B)
                err_m = sp.tile([1, 1], F32, tag="err_m")
                nc.vector.tensor_scalar(out=err_m, in0=cc_res[:, 0:1],
                                        scalar1=inv_b, scalar2=None, op0=ALU.mult)
                probs_m = sp.tile([1, NL], F32, tag="probs_m")
                nc.vector.tensor_scalar(out=probs_m, in0=cc_res[:, 1:1 + NL],
                                        scalar1=inv_b, scalar2=None, op0=ALU.mult)
                mxp = sp.tile([1, 1], F32, tag="mxp")
                nc.vector.reduce_max(out=mxp, in_=probs_m, axis=AX.X)
                msk = sp.tile([1, NL], F32, tag="msk")
                nc.vector.tensor_scalar(out=msk, in0=probs_m, scalar1=mxp[:, 0:1],
                                        scalar2=None, op0=ALU.is_ge)
                mi = sp.tile([1, NL], F32, tag="mi")
                nc.vector.tensor_mul(mi, msk, iota_f)
                m2 = sp.tile([1, NL], F32, tag="m2")
                nc.vector.tensor_scalar(out=m2, in0=msk, scalar1=-1e9, scalar2=1e9,
                                        op0=ALU.mult, op1=ALU.add)
                nc.vector.tensor_tensor(out=mi, in0=mi, in1=m2, op=ALU.add)
                mlev = sp.tile([1, 1], F32, tag="mlev")
                nc.vector.tensor_reduce(out=mlev, in_=mi, axis=AX.X, op=ALU.min)
                # level = c1?2 : (c2?max_level : (c3?0 : 3))
                c1 = sp.tile([1, 1], F32, tag="c1")
                nc.vector.tensor_scalar(out=c1, in0=err_m, scalar1=CUTOFF,
                                        scalar2=None, op0=ALU.is_gt)
                c2 = sp.tile([1, 1], F32, tag="c2")
                nc.vector.tensor_scalar(out=c2, in0=mxp, scalar1=0.7,
                                        scalar2=None, op0=ALU.is_gt)
                c3 = sp.tile([1, 1], F32, tag="c3")
                nc.vector.tensor_scalar(out=c3, in0=err_m, scalar1=0.5,
                                        scalar2=None, op0=ALU.is_lt)
                inner = sp.tile([1, 1], F32, tag="inner")
                nc.vector.tensor_scalar(out=inner, in0=c3, scalar1=-3.0, scalar2=3.0,
                                        op0=ALU.mult, op1=ALU.add)
                mid = sp.tile([1, 1], F32, tag="mid")
                nc.vector.tensor_mul(mid, c2, mlev)
                nc2 = sp.tile([1, 1], F32, tag="nc2")
                nc.vector.tensor_scalar(out=nc2, in0=c2, scalar1=-1.0, scalar2=1.0,
                                        op0=ALU.mult, op1=ALU.add)
                nc.vector.tensor_tensor(out=nc2, in0=nc2, in1=inner, op=ALU.mult)
                nc.vector.tensor_tensor(out=mid, in0=mid, in1=nc2, op=ALU.add)
                lev = sp.tile([1, 1], F32, tag="lev")
                nc.vector.tensor_scalar(out=lev, in0=c1, scalar1=2.0,
                                        scalar2=None, op0=ALU.mult)
                nc1 = sp.tile([1, 1], F32, tag="nc1")
                nc.vector.tensor_scalar(out=nc1, in0=c1, scalar1=-1.0, scalar2=1.0,
                                        op0=ALU.mult, op1=ALU.add)
                nc.vector.tensor_tensor(out=nc1, in0=nc1, in1=mid, op=ALU.mult)
                nc.vector.tensor_tensor(out=lev, in0=lev, in1=nc1, op=ALU.add)
                # a = 1 - 0.1*eq0 - 0.3*eq1 ; b = 0.1*eq0 + 0.3*eq1
                eq0 = sp.tile([1, 1], F32, tag="eq0")
                nc.vector.tensor_scalar(out=eq0, in0=lev, scalar1=0.0,
                                        scalar2=None, op0=ALU.is_equal)
                eq1 = sp.tile([1, 1], F32, tag="eq1")
                nc.vector.tensor_scalar(out=eq1, in0=lev, scalar1=1.0,
                                        scalar2=None, op0=ALU.is_equal)
                bmix = sp.tile([1, 1], F32, tag="bmix")
                nc.vector.tensor_scalar(out=bmix, in0=eq1, scalar1=0.3,
                                        scalar2=None, op0=ALU.mult)
                nc.vector.scalar_tensor_tensor(out=bmix, in0=eq0,
                                               scalar=float(1.0 - FADE), in1=bmix,
                                               op0=ALU.mult, op1=ALU.add)
                amix = sp.tile([1, 1], F32, tag="amix")
                nc.vector.tensor_scalar(out=amix, in0=bmix, scalar1=-1.0, scalar2=1.0,
                                        op0=ALU.mult, op1=ALU.add)
                lvl_i = sp.tile([1, 1], I32, tag="lvl_i")
                nc.vector.tensor_copy(out=lvl_i, in_=lev)
                nc.sync.dma_start(out=lvl_d[:, :], in_=lvl_i)
                # broadcast a to all partitions
                ab_row = sp.tile([1, 2], BF16, tag="ab_row")
                nc.vector.tensor_copy(out=ab_row[:, 0:1], in_=amix)
                nc.vector.tensor_copy(out=ab_row[:, 1:2], in_=bmix)
                ps_ab = ps_ms.tile([P, 2], F32, tag="ps")
                nc.tensor.matmul(ps_ab, ones_row, ab_row, start=True, stop=True)
                ab_b = sp.tile([P, 2], F32, tag="ab_b")
                nc.vector.tensor_copy(out=ab_b, in_=ps_ab)
                # hub row scaled by b, in place (hubr not needed afterwards)
                nc.vector.tensor_scalar(out=hubr, in0=hubr, scalar1=bmix[:, 0:1],
                                        scalar2=None, op0=ALU.mult)
                # debug stats
                stt = sp.tile([1, 16], F32, tag="stt")
                nc.vector.memset(stt, 0.0)
                for i, t in enumerate((err_m, mxp, mlev, lev, amix, bmix, c_t, err_p)):
                    nc.vector.tensor_copy(out=stt[:, i:i + 1], in_=t)
                nc.vector.tensor_copy(out=stt[:, 8:8 + NL], in_=probs_m)
                nc.sync.dma_start(out=st_d[:, :], in_=stt)

            # ---------------- phase: final mix + output ----------------
            with nc.named_scope("final"):
                for bt in range(NBT):
                    for ch in range(NCH):
                        ps_hub = ps_h1.tile([P, CH], F32, tag="h1")
                        nc.tensor.matmul(ps_hub, ones_row, hubr[:, ts(ch, CH)],
                                         start=True, stop=True)
                        ot = tp.tile([P, CH], F32, tag="ot")
                        nc.vector.scalar_tensor_tensor(
                            out=ot, in0=acc[bt][:, ts(ch, CH)],
                            scalar=ab_b[:, 0:1], in1=ps_hub,
                            op0=ALU.mult, op1=ALU.add)
                        nc.sync.dma_start(out=out_d[ts(bt, P), ts(ch, CH)], in_=ot)

    nc.compile()
    return nc


# ----------------------------------------------------------------------------
# host side
# ----------------------------------------------------------------------------
_PROGRAM_CACHE = {}


def _get_program(key, **kw):
    if key not in _PROGRAM_CACHE:
        _PROGRAM_CACHE[key] = build_program(**kw)
    return _PROGRAM_CACHE[key]


def _prep_inputs(input_data, hub_content, Ws, bs, Wg, bg, Wl1, bl1, Wl2, bl2,
                 We1, be1, We2, be2):
    bf = ml_dtypes.bfloat16
    f32 = np.float32
    B, D = input_data.shape
    E = Ws.shape[0]
    DL1 = Wl1.shape[0]
    DE1 = We1.shape[0]
    NL = Wl2.shape[0]
    SH = B // NCORES
    NJT = DL1 // P

    wsT = np.ascontiguousarray(np.transpose(Ws, (0, 2, 1))).astype(bf)
    v = np.einsum("eod,eo->de", Ws.astype(np.float64),
                  Wg.astype(np.float64)).astype(f32).astype(bf)
    ce = (np.sum(bs.astype(np.float64) * Wg.astype(np.float64), axis=1)
          + bg.astype(np.float64)).astype(f32).reshape(1, E).astype(bf)
    shared = {
        "wsT": wsT,
        "v": np.ascontiguousarray(v),
        "ce": ce,
        "bs": bs.astype(bf),
        "wl1T": np.ascontiguousarray(Wl1.T).astype(bf),
        "wl2T": np.ascontiguousarray(Wl2.T).astype(bf),
        "bl1r": np.ascontiguousarray(bl1.reshape(NJT, P).T).astype(f32),
        "bl2r": bl2.reshape(1, NL).astype(bf),
        "we1T": np.ascontiguousarray(We1.T).astype(bf),
        "we2r": We2.reshape(1, DE1).astype(bf),
        "be1r": be1.reshape(1, DE1).astype(f32),
        "be2s": be2.reshape(1, 1).astype(f32),
        "hubr": hub_content.reshape(1, D).astype(bf),
        "hubT": np.ascontiguousarray(hub_content.reshape(1, D).T).astype(bf),
    }
    in_maps = []
    for i in range(NCORES):
        shard = input_data[i * SH:(i + 1) * SH]
        xT = np.ascontiguousarray(shard.T).astype(bf)
        in_maps.append({"xT": xT, **shared})
    return in_maps


def run(inputs, trace=False):
    inputs = {k: np.asarray(v) for k, v in inputs.items()}
    B, D = inputs["input_data"].shape
    E = inputs["Ws"].shape[0]
    DL1 = inputs["Wl1"].shape[0]
    DE1 = inputs["We1"].shape[0]
    NL = inputs["Wl2"].shape[0]
    key = (B, D, E, DL1, DE1, NL)
    nc = _get_program(key, B=B, D=D, E=E, DL1=DL1, DE1=DE1, NL=NL)
    in_maps = _prep_inputs(**inputs)
    res = run_bass_kernel_spmd(nc, in_maps, list(range(NCORES)), trace=trace)
    output = np.concatenate([res.results[i]["out"] for i in range(NCORES)], axis=0)
    level = np.int32(res.results[0]["lvl"][0, 0]).reshape(())
    return output, level, res


def kernel(**inputs):
    output, level, _ = run(inputs)
    return output, level


if __name__ == "__main__":
    pass
